# revision 4
# baseline (speedup 1.0000x reference)
"""Compact Bilinear Pooling (count-sketch + FFT circular correlation) as a
Trainium2 Bass kernel, data-parallel over batch across 8 NeuronCores.

Math: FFT(count_sketch(x; s, h))[k] = sum_c x[c] * s[c] * exp(-2pi i h[c] k / D)
    = x @ A, a dense complex matrix built on the host from (s, h). So the whole
layer is: Y1 = X1 @ A1, Y2 = X2 @ A2 (per-row half spectra), elementwise
complex product, sum-pool over the 14x14 window (via a 0/1 pooling matmul),
then a real inverse FFT of the pooled [4, D] spectrum per core, done as a
two-stage Cooley-Tukey factorization (D = 64*128) of small matmuls.

All matmuls run as float32r (TF32-like, 1 cycle/row on the PE).
"""
import numpy as np

import concourse.bass as bass
import concourse.tile as tile
from concourse import bacc, mybir
from concourse.bass_utils import run_bass_kernel_spmd

B, Hh, Ww, C, D = 32, 14, 14, 512, 8192
NCORES = 8
BPC = B // NCORES        # 4 batches per core
HW = Hh * Ww             # 196
ROWS = BPC * HW          # 784 rows per core
RT = 7                   # row tiles of 128
ROWS_PAD = RT * 128      # 896
KC, KCW = 9, 456         # frequency chunks
KP = KC * KCW            # 4104 >= D/2 + 1
CCN = 4                  # contraction chunks (C = 4*128)

F32 = mybir.dt.float32
F32R = mybir.dt.float32r


def _round_fp32r(x: np.ndarray) -> np.ndarray:
    """Round-to-nearest-even dropping the low 12 mantissa bits (measured
    float32r behaviour of the DVE rounding path on trn2)."""
    b = np.ascontiguousarray(x, dtype=np.float32).view(np.uint32)
    r = (b + np.uint32(0x7FF) + ((b >> np.uint32(12)) & np.uint32(1))) & np.uint32(0xFFFFF000)
    return r.view(np.float32)


def _build_nc():
    nc = bacc.Bacc("TRN2", target_bir_lowering=False)

    xt_d = nc.dram_tensor("xt", [128, 2, CCN, RT, 128], F32R, kind="ExternalInput")
    amat_d = nc.dram_tensor("amat", [128, 4, CCN, KC, KCW], F32R, kind="ExternalInput")
    gp_d = nc.dram_tensor("gpool", [128, 3, RT, 8], F32R, kind="ExternalInput")
    w1_d = nc.dram_tensor("w1", [128, 3, 128], F32R, kind="ExternalInput")
    w2_d = nc.dram_tensor("w2", [64, 2, 64], F32R, kind="ExternalInput")
    tw_d = nc.dram_tensor("tw", [64, 2, 128], F32, kind="ExternalInput")
    id_d = nc.dram_tensor("ident", [128, 128], F32, kind="ExternalInput")
    out_d = nc.dram_tensor("out", [BPC, D], F32, kind="ExternalOutput")

    with tile.TileContext(nc) as tc:
        with tc.tile_pool(name="const", bufs=1) as pc, \
             tc.tile_pool(name="astream", bufs=2) as pa, \
             tc.tile_pool(name="work", bufs=2) as pw, \
             tc.tile_pool(name="qstage", bufs=2) as pqs, \
             tc.tile_pool(name="dram", bufs=1, space="DRAM") as pd:

            xt = pc.tile([128, 2, CCN, RT, 128], F32R)
            nc.sync.dma_start(xt, xt_d[:, :, :, :, :])
            gp = pc.tile([128, 3, RT, 8], F32R)
            nc.sync.dma_start(gp, gp_d[:, :, :, :])

            qdram = pd.tile([8, D], F32R)
            # zero-fill the padded tail of the spectrum
            zs = pc.tile([8, D - KP], F32)
            nc.vector.memset(zs, 0.0)
            nc.sync.dma_start(qdram[:, KP:], zs.bitcast(F32R))

            # ---------------- main loop ----------------
            with tc.tile_pool(name="py1", bufs=1, space="PSUM") as py1, \
                 tc.tile_pool(name="py2", bufs=2, space="PSUM") as py2, \
                 tc.tile_pool(name="pq", bufs=2, space="PSUM") as pq:
                for kc in range(KC):
                    at = pa.tile([128, 4, CCN, KCW], F32R, tag="amat")
                    nc.sync.dma_start(at, amat_d[:, :, :, kc, :])
                    qps = pq.tile([8, KCW], F32, tag="qps")
                    for rt in range(RT):
                        yps = {}
                        for t in range(4):
                            pool = py1 if t < 2 else py2
                            yps[t] = pool.tile([128, KCW], F32, tag=f"y{t}", name=f"y{t}")
                        for cc in range(CCN):
                            for inp in range(2):
                                for ri in range(2):
                                    t = inp * 2 + ri
                                    nc.tensor.matmul(
                                        yps[t],
                                        xt[:, inp, cc, rt, :],
                                        at[:, t, cc, :],
                                        start=(cc == 0),
                                        stop=(cc == CCN - 1),
                                    )
                        y1r_sb = pw.tile([128, KCW], F32, tag="y1r_sb")
                        y1i_sb = pw.tile([128, KCW], F32, tag="y1i_sb")
                        nc.scalar.copy(y1r_sb, yps[0])
                        nc.scalar.copy(y1i_sb, yps[1])
                        u = pw.tile([128, KCW], F32R, tag="u")
                        v = pw.tile([128, KCW], F32R, tag="v")
                        t1 = pw.tile([128, KCW], F32R, tag="t1")
                        t2 = pw.tile([128, KCW], F32R, tag="t2")
                        nc.vector.tensor_mul(u, y1r_sb, yps[2])
                        nc.vector.tensor_mul(v, y1i_sb, yps[3])
                        nc.vector.tensor_mul(t1, y1r_sb, yps[3])
                        nc.vector.tensor_mul(t2, y1i_sb, yps[2])
                        first, last = (rt == 0), (rt == RT - 1)
                        nc.tensor.matmul(qps, gp[:, 0, rt, :], u,
                                         start=first, stop=False)
                        nc.tensor.matmul(qps, gp[:, 1, rt, :], v,
                                         start=False, stop=False)
                        nc.tensor.matmul(qps, gp[:, 2, rt, :], t1,
                                         start=False, stop=False)
                        nc.tensor.matmul(qps, gp[:, 2, rt, :], t2,
                                         start=False, stop=last)
                    qst = pqs.tile([8, KCW], F32R, tag="qst")
                    nc.vector.tensor_copy(qst, qps)
                    nc.sync.dma_start(qdram[:, kc * KCW:(kc + 1) * KCW], qst)

            # ---------------- inverse FFT tail ----------------
            w1 = pc.tile([128, 3, 128], F32R)
            nc.sync.dma_start(w1, w1_d[:, :, :])
            w2 = pc.tile([64, 2, 64], F32R)
            nc.sync.dma_start(w2, w2_d[:, :, :])
            tw = pc.tile([64, 2, 128], F32)
            nc.sync.dma_start(tw, tw_d[:, :, :])
            ident = pc.tile([128, 128], F32)
            nc.sync.dma_start(ident, id_d[:, :])

            with tc.tile_pool(name="pif", bufs=1, space="PSUM") as pif, \
                 tc.tile_pool(name="ptr", bufs=2, space="PSUM") as ptr, \
                 tc.tile_pool(name="pifs", bufs=1) as pifs, \
                 tc.tile_pool(name="ptmp", bufs=2) as ptmp:
                # reshape DMA: Qc as [a=128, b=4, r=64]
                qar = pifs.tile([128, BPC * 64], F32R, tag="qar")
                qai = pifs.tile([128, BPC * 64], F32R, tag="qai")
                nc.sync.dma_start(qar.rearrange("p (b r) -> p b r", r=64),
                                  qdram[0:BPC].rearrange("b (a r) -> a b r", r=64))
                nc.sync.dma_start(qai.rearrange("p (b r) -> p b r", r=64),
                                  qdram[BPC:2 * BPC].rearrange("b (a r) -> a b r", r=64))

                # stage 1: V[q, (b,r)] ; contraction over a
                vr_ps = pif.tile([128, BPC * 64], F32, tag="vr")
                vi_ps = pif.tile([128, BPC * 64], F32, tag="vi")
                nc.tensor.matmul(vr_ps, w1[:, 0, :], qar, start=True, stop=False)
                nc.tensor.matmul(vr_ps, w1[:, 2, :], qai, start=False, stop=True)
                nc.tensor.matmul(vi_ps, w1[:, 1, :], qar, start=True, stop=False)
                nc.tensor.matmul(vi_ps, w1[:, 0, :], qai, start=False, stop=True)
                vr_sb = pifs.tile([128, BPC * 64], F32, tag="vr_sb")
                vi_sb = pifs.tile([128, BPC * 64], F32, tag="vi_sb")
                nc.scalar.copy(vr_sb, vr_ps)
                nc.scalar.copy(vi_sb, vi_ps)

                # per-b transpose [128 q, 64 r] -> [64 r, 128 q], then twiddle
                tr_sb = pifs.tile([64, BPC * 128], F32R, tag="tr_sb")
                ti_sb = pifs.tile([64, BPC * 128], F32R, tag="ti_sb")
                for b in range(BPC):
                    trp = ptr.tile([64, 128], F32, tag="trp")
                    tip = ptr.tile([64, 128], F32, tag="tip")
                    nc.tensor.transpose(trp, vr_sb[:, b * 64:(b + 1) * 64], ident)
                    nc.tensor.transpose(tip, vi_sb[:, b * 64:(b + 1) * 64], ident)
                    m1 = ptmp.tile([64, 128], F32, tag="m1")
                    m2 = ptmp.tile([64, 128], F32, tag="m2")
                    m3 = ptmp.tile([64, 128], F32, tag="m3")
                    m4 = ptmp.tile([64, 128], F32, tag="m4")
                    nc.vector.tensor_mul(m1, trp, tw[:, 0, :])
                    nc.vector.tensor_mul(m2, tip, tw[:, 1, :])
                    nc.vector.tensor_mul(m3, trp, tw[:, 1, :])
                    nc.vector.tensor_mul(m4, tip, tw[:, 0, :])
                    nc.vector.tensor_sub(tr_sb[:, b * 128:(b + 1) * 128], m1, m2)
                    nc.vector.tensor_add(ti_sb[:, b * 128:(b + 1) * 128], m3, m4)

                # stage 2: out[t, (b,q)] = c2^T Tr + (-s2)^T Ti
                ops = pif.tile([64, BPC * 128], F32, tag="ops")
                nc.tensor.matmul(ops, w2[:, 0, :], tr_sb, start=True, stop=False)
                nc.tensor.matmul(ops, w2[:, 1, :], ti_sb, start=False, stop=True)
                res = pifs.tile([64, BPC * 128], F32, tag="res")
                nc.scalar.copy(res, ops)
                for b in range(BPC):
                    nc.sync.dma_start(
                        out_d[b].rearrange("(t q) -> t q", q=128),
                        res[:, b * 128:(b + 1) * 128])

    nc.compile()
    return nc


def _host_consts(rand_s_1, rand_s_2, rand_h_1, rand_h_2):
    k = np.arange(KP)
    alpha = np.where((k == 0) | (k == D // 2), 1.0, 2.0) / D
    alpha = np.where(k > D // 2, 0.0, alpha)
    live = (k <= D // 2).astype(np.float64)
    s1 = rand_s_1.astype(np.float64)
    s2 = rand_s_2.astype(np.float64)
    th1 = 2.0 * np.pi * ((rand_h_1.astype(np.int64)[:, None] * k[None, :]) % D) / D
    th2 = 2.0 * np.pi * ((rand_h_2.astype(np.int64)[:, None] * k[None, :]) % D) / D
    A = np.empty((4, C, KP), np.float32)
    A[0] = s1[:, None] * np.cos(th1) * alpha
    A[1] = -s1[:, None] * np.sin(th1) * alpha
    A[2] = s2[:, None] * np.cos(th2) * live
    A[3] = -s2[:, None] * np.sin(th2) * live
    # amat layout [p, tensor, cc, kc, kcw]
    amat = np.ascontiguousarray(
        A.reshape(4, CCN, 128, KC, KCW).transpose(2, 0, 1, 3, 4))
    amat = _round_fp32r(amat)

    # pooling matrices [p, ver, rt, 8]
    gp = np.zeros((RT, 128, 3, 8), np.float32)
    for rt in range(RT):
        for p in range(128):
            r_ = rt * 128 + p
            if r_ < ROWS:
                b = r_ // HW
                gp[rt, p, 0, b] = 1.0
                gp[rt, p, 1, b] = -1.0
                gp[rt, p, 2, 4 + b] = 1.0
    gp = np.ascontiguousarray(gp.transpose(1, 2, 0, 3))

    a = np.arange(128)[:, None]
    q = np.arange(128)[None, :]
    c1 = np.cos(2 * np.pi * a * q / 128)
    s1m = np.sin(2 * np.pi * a * q / 128)
    w1 = np.stack([c1, s1m, -s1m], 1).astype(np.float32)  # [128, 3, 128]
    r_ = np.arange(64)[:, None]
    t_ = np.arange(64)[None, :]
    c2 = np.cos(2 * np.pi * t_ * r_ / 64)
    s2m = np.sin(2 * np.pi * t_ * r_ / 64)
    w2 = np.stack([c2, -s2m], 1).astype(np.float32)       # [64, 2, 64]
    ctw = np.cos(2 * np.pi * q * r_ / D)
    stw = np.sin(2 * np.pi * q * r_ / D)
    tw = np.stack([ctw, stw], 1).astype(np.float32)       # [64, 2, 128]
    ident = np.eye(128, dtype=np.float32)
    return amat, gp, _round_fp32r(w1), _round_fp32r(w2), tw, ident


_NC_CACHE = None
LAST_RESULTS = None


def kernel(bottom1, bottom2, rand_s_1, rand_s_2, rand_h_1, rand_h_2):
    global _NC_CACHE
    if _NC_CACHE is None:
        _NC_CACHE = _build_nc()
    nc = _NC_CACHE

    amat, gp, w1, w2, tw, ident = _host_consts(
        np.asarray(rand_s_1), np.asarray(rand_s_2),
        np.asarray(rand_h_1), np.asarray(rand_h_2))

    x1 = np.asarray(bottom1, np.float32).reshape(B, HW, C)
    x2 = np.asarray(bottom2, np.float32).reshape(B, HW, C)

    in_maps = []
    for core in range(NCORES):
        bs = slice(core * BPC, (core + 1) * BPC)
        xt = np.zeros((2, C, ROWS_PAD), np.float32)
        xt[0, :, :ROWS] = x1[bs].reshape(ROWS, C).T
        xt[1, :, :ROWS] = x2[bs].reshape(ROWS, C).T
        # layout [p, inp, cc, rt, 128]
        xt = np.ascontiguousarray(
            xt.reshape(2, CCN, 128, RT, 128).transpose(2, 0, 1, 3, 4))
        xt = _round_fp32r(xt)
        in_maps.append({
            "xt": xt, "amat": amat, "gpool": gp,
            "w1": w1, "w2": w2, "tw": tw, "ident": ident,
        })

    res = run_bass_kernel_spmd(nc, in_maps, core_ids=list(range(NCORES)))
    global LAST_RESULTS
    LAST_RESULTS = res
    out = np.concatenate([res.results[c]["out"] for c in range(NCORES)], 0)
    return out.astype(np.float32)


if __name__ == "__main__":
    rng = np.random.default_rng(0)
    b1 = rng.standard_normal((B, Hh, Ww, C)).astype(np.float32)
    b2 = rng.standard_normal((B, Hh, Ww, C)).astype(np.float32)
    s1 = (2.0 * rng.integers(0, 2, C) - 1.0).astype(np.float32)
    s2 = (2.0 * rng.integers(0, 2, C) - 1.0).astype(np.float32)
    h1 = rng.integers(0, D, C, dtype=np.int32)
    h2 = rng.integers(0, D, C, dtype=np.int32)
    out = kernel(bottom1=b1, bottom2=b2, rand_s_1=s1, rand_s_2=s2,
                 rand_h_1=h1, rand_h_2=h2)
    print(out.shape, out.dtype)


# revision 5
# speedup vs baseline: 1.0384x; 1.0384x over previous
"""Compact Bilinear Pooling (count-sketch + FFT circular correlation) as a
Trainium2 Bass kernel, data-parallel over batch across 8 NeuronCores.

Math: FFT(count_sketch(x; s, h))[k] = sum_c x[c] * s[c] * exp(-2pi i h[c] k / D)
    = x @ A, a dense complex matrix built on the host from (s, h). So the whole
layer is: Y1 = X1 @ A1, Y2 = X2 @ A2 (per-row half spectra), elementwise
complex product, sum-pool over the 14x14 window (via a 0/1 pooling matmul),
then a real inverse FFT of the pooled [4, D] spectrum per core, done as a
two-stage Cooley-Tukey factorization (D = 64*128) of small matmuls.

All matmuls run as float32r (TF32-like, 1 cycle/row on the PE).
"""
import numpy as np

import concourse.bass as bass
import concourse.tile as tile
from concourse import bacc, mybir
from concourse.bass_utils import run_bass_kernel_spmd

B, Hh, Ww, C, D = 32, 14, 14, 512, 8192
NCORES = 8
BPC = B // NCORES        # 4 batches per core
HW = Hh * Ww             # 196
ROWS = BPC * HW          # 784 rows per core
RT = 7                   # row tiles of 128
ROWS_PAD = RT * 128      # 896
KC, KCW = 9, 456         # frequency chunks
KP = KC * KCW            # 4104 >= D/2 + 1
CCN = 4                  # contraction chunks (C = 4*128)

F32 = mybir.dt.float32
F32R = mybir.dt.float32r


def _round_fp32r(x: np.ndarray) -> np.ndarray:
    """Round-to-nearest-even dropping the low 12 mantissa bits (measured
    float32r behaviour of the DVE rounding path on trn2)."""
    b = np.ascontiguousarray(x, dtype=np.float32).view(np.uint32)
    r = (b + np.uint32(0x7FF) + ((b >> np.uint32(12)) & np.uint32(1))) & np.uint32(0xFFFFF000)
    return r.view(np.float32)


def _build_nc():
    nc = bacc.Bacc("TRN2", target_bir_lowering=False)

    xt_d = nc.dram_tensor("xt", [128, RT, 2, CCN, 128], F32R, kind="ExternalInput")
    amat_d = nc.dram_tensor("amat", [128, 4, CCN, KC, KCW], F32R, kind="ExternalInput")
    gp_d = nc.dram_tensor("gpool", [128, 3, RT, 8], F32R, kind="ExternalInput")
    w1_d = nc.dram_tensor("w1", [128, 3, 128], F32R, kind="ExternalInput")
    w2_d = nc.dram_tensor("w2", [64, 2, 64], F32R, kind="ExternalInput")
    tw_d = nc.dram_tensor("tw", [64, 2, 128], F32, kind="ExternalInput")
    id_d = nc.dram_tensor("ident", [128, 128], F32, kind="ExternalInput")
    out_d = nc.dram_tensor("out", [BPC, D], F32, kind="ExternalOutput")

    with tile.TileContext(nc) as tc:
        with tc.tile_pool(name="const", bufs=1) as pc, \
             tc.tile_pool(name="astream", bufs=2) as pa, \
             tc.tile_pool(name="work", bufs=2) as pw, \
             tc.tile_pool(name="qstage", bufs=2) as pqs, \
             tc.tile_pool(name="dram", bufs=1, space="DRAM") as pd:

            xt = pc.tile([128, RT, 2, CCN, 128], F32R)
            for rt in range(RT):
                nc.sync.dma_start(xt[:, rt], xt_d[:, rt])
            gp = pc.tile([128, 3, RT, 8], F32R)
            nc.sync.dma_start(gp, gp_d[:, :, :, :])

            qdram = pd.tile([8, D], F32R)
            # zero-fill the padded tail of the spectrum
            zs = pc.tile([8, D - KP], F32)
            nc.vector.memset(zs, 0.0)
            nc.sync.dma_start(qdram[:, KP:], zs.bitcast(F32R))

            # ---------------- main loop ----------------
            with tc.tile_pool(name="py1", bufs=1, space="PSUM") as py1, \
                 tc.tile_pool(name="py2", bufs=2, space="PSUM") as py2, \
                 tc.tile_pool(name="pq", bufs=2, space="PSUM") as pq:
                for kc in range(KC):
                    at = pa.tile([128, 4, CCN, KCW], F32R, tag="amat")
                    for t_ in range(4):
                        nc.sync.dma_start(at[:, t_], amat_d[:, t_, :, kc, :])
                    qps = pq.tile([8, KCW], F32, tag="qps")
                    for rt in range(RT):
                        yps = {}
                        for t in range(4):
                            pool = py1 if t < 2 else py2
                            yps[t] = pool.tile([128, KCW], F32, tag=f"y{t}", name=f"y{t}")
                        for cc in range(CCN):
                            for inp in range(2):
                                for ri in range(2):
                                    t = inp * 2 + ri
                                    nc.tensor.matmul(
                                        yps[t],
                                        xt[:, rt, inp, cc, :],
                                        at[:, t, cc, :],
                                        start=(cc == 0),
                                        stop=(cc == CCN - 1),
                                    )
                        y1r_sb = pw.tile([128, KCW], F32, tag="y1r_sb")
                        y1i_sb = pw.tile([128, KCW], F32, tag="y1i_sb")
                        nc.scalar.copy(y1r_sb, yps[0])
                        nc.scalar.copy(y1i_sb, yps[1])
                        u = pw.tile([128, KCW], F32R, tag="u")
                        v = pw.tile([128, KCW], F32R, tag="v")
                        t1 = pw.tile([128, KCW], F32R, tag="t1")
                        t2 = pw.tile([128, KCW], F32R, tag="t2")
                        nc.vector.tensor_mul(u, y1r_sb, yps[2])
                        nc.vector.tensor_mul(v, y1i_sb, yps[3])
                        nc.vector.tensor_mul(t1, y1r_sb, yps[3])
                        nc.vector.tensor_mul(t2, y1i_sb, yps[2])
                        first, last = (rt == 0), (rt == RT - 1)
                        nc.tensor.matmul(qps, gp[:, 0, rt, :], u,
                                         start=first, stop=False)
                        nc.tensor.matmul(qps, gp[:, 1, rt, :], v,
                                         start=False, stop=False)
                        nc.tensor.matmul(qps, gp[:, 2, rt, :], t1,
                                         start=False, stop=False)
                        nc.tensor.matmul(qps, gp[:, 2, rt, :], t2,
                                         start=False, stop=last)
                    qst = pqs.tile([8, KCW], F32R, tag="qst")
                    nc.vector.tensor_copy(qst, qps)
                    nc.sync.dma_start(qdram[:, kc * KCW:(kc + 1) * KCW], qst)

            # ---------------- inverse FFT tail ----------------
            w1 = pc.tile([128, 3, 128], F32R)
            nc.sync.dma_start(w1, w1_d[:, :, :])
            w2 = pc.tile([64, 2, 64], F32R)
            nc.sync.dma_start(w2, w2_d[:, :, :])
            tw = pc.tile([64, 2, 128], F32)
            nc.sync.dma_start(tw, tw_d[:, :, :])
            ident = pc.tile([128, 128], F32)
            nc.sync.dma_start(ident, id_d[:, :])

            with tc.tile_pool(name="pif", bufs=1, space="PSUM") as pif, \
                 tc.tile_pool(name="ptr", bufs=2, space="PSUM") as ptr, \
                 tc.tile_pool(name="pifs", bufs=1) as pifs, \
                 tc.tile_pool(name="ptmp", bufs=2) as ptmp:
                # reshape DMA: Qc as [a=128, b=4, r=64]
                qar = pifs.tile([128, BPC * 64], F32R, tag="qar")
                qai = pifs.tile([128, BPC * 64], F32R, tag="qai")
                nc.sync.dma_start(qar.rearrange("p (b r) -> p b r", r=64),
                                  qdram[0:BPC].rearrange("b (a r) -> a b r", r=64))
                nc.sync.dma_start(qai.rearrange("p (b r) -> p b r", r=64),
                                  qdram[BPC:2 * BPC].rearrange("b (a r) -> a b r", r=64))

                # stage 1: V[q, (b,r)] ; contraction over a
                vr_ps = pif.tile([128, BPC * 64], F32, tag="vr")
                vi_ps = pif.tile([128, BPC * 64], F32, tag="vi")
                nc.tensor.matmul(vr_ps, w1[:, 0, :], qar, start=True, stop=False)
                nc.tensor.matmul(vr_ps, w1[:, 2, :], qai, start=False, stop=True)
                nc.tensor.matmul(vi_ps, w1[:, 1, :], qar, start=True, stop=False)
                nc.tensor.matmul(vi_ps, w1[:, 0, :], qai, start=False, stop=True)
                vr_sb = pifs.tile([128, BPC * 64], F32, tag="vr_sb")
                vi_sb = pifs.tile([128, BPC * 64], F32, tag="vi_sb")
                nc.scalar.copy(vr_sb, vr_ps)
                nc.scalar.copy(vi_sb, vi_ps)

                # per-b transpose [128 q, 64 r] -> [64 r, 128 q], then twiddle
                tr_sb = pifs.tile([64, BPC * 128], F32R, tag="tr_sb")
                ti_sb = pifs.tile([64, BPC * 128], F32R, tag="ti_sb")
                for b in range(BPC):
                    trp = ptr.tile([64, 128], F32, tag="trp")
                    tip = ptr.tile([64, 128], F32, tag="tip")
                    nc.tensor.transpose(trp, vr_sb[:, b * 64:(b + 1) * 64], ident)
                    nc.tensor.transpose(tip, vi_sb[:, b * 64:(b + 1) * 64], ident)
                    m1 = ptmp.tile([64, 128], F32, tag="m1")
                    m2 = ptmp.tile([64, 128], F32, tag="m2")
                    m3 = ptmp.tile([64, 128], F32, tag="m3")
                    m4 = ptmp.tile([64, 128], F32, tag="m4")
                    nc.vector.tensor_mul(m1, trp, tw[:, 0, :])
                    nc.vector.tensor_mul(m2, tip, tw[:, 1, :])
                    nc.vector.tensor_mul(m3, trp, tw[:, 1, :])
                    nc.vector.tensor_mul(m4, tip, tw[:, 0, :])
                    nc.vector.tensor_sub(tr_sb[:, b * 128:(b + 1) * 128], m1, m2)
                    nc.vector.tensor_add(ti_sb[:, b * 128:(b + 1) * 128], m3, m4)

                # stage 2: out[t, (b,q)] = c2^T Tr + (-s2)^T Ti
                ops = pif.tile([64, BPC * 128], F32, tag="ops")
                nc.tensor.matmul(ops, w2[:, 0, :], tr_sb, start=True, stop=False)
                nc.tensor.matmul(ops, w2[:, 1, :], ti_sb, start=False, stop=True)
                res = pifs.tile([64, BPC * 128], F32, tag="res")
                nc.scalar.copy(res, ops)
                for b in range(BPC):
                    nc.sync.dma_start(
                        out_d[b].rearrange("(t q) -> t q", q=128),
                        res[:, b * 128:(b + 1) * 128])

    nc.compile()
    return nc


def _host_consts(rand_s_1, rand_s_2, rand_h_1, rand_h_2):
    k = np.arange(KP)
    alpha = np.where((k == 0) | (k == D // 2), 1.0, 2.0) / D
    alpha = np.where(k > D // 2, 0.0, alpha)
    live = (k <= D // 2).astype(np.float64)
    s1 = rand_s_1.astype(np.float64)
    s2 = rand_s_2.astype(np.float64)
    th1 = 2.0 * np.pi * ((rand_h_1.astype(np.int64)[:, None] * k[None, :]) % D) / D
    th2 = 2.0 * np.pi * ((rand_h_2.astype(np.int64)[:, None] * k[None, :]) % D) / D
    A = np.empty((4, C, KP), np.float32)
    A[0] = s1[:, None] * np.cos(th1) * alpha
    A[1] = -s1[:, None] * np.sin(th1) * alpha
    A[2] = s2[:, None] * np.cos(th2) * live
    A[3] = -s2[:, None] * np.sin(th2) * live
    # amat layout [p, tensor, cc, kc, kcw]
    amat = np.ascontiguousarray(
        A.reshape(4, CCN, 128, KC, KCW).transpose(2, 0, 1, 3, 4))
    amat = _round_fp32r(amat)

    # pooling matrices [p, ver, rt, 8]
    gp = np.zeros((RT, 128, 3, 8), np.float32)
    for rt in range(RT):
        for p in range(128):
            r_ = rt * 128 + p
            if r_ < ROWS:
                b = r_ // HW
                gp[rt, p, 0, b] = 1.0
                gp[rt, p, 1, b] = -1.0
                gp[rt, p, 2, 4 + b] = 1.0
    gp = np.ascontiguousarray(gp.transpose(1, 2, 0, 3))

    a = np.arange(128)[:, None]
    q = np.arange(128)[None, :]
    c1 = np.cos(2 * np.pi * a * q / 128)
    s1m = np.sin(2 * np.pi * a * q / 128)
    w1 = np.stack([c1, s1m, -s1m], 1).astype(np.float32)  # [128, 3, 128]
    r_ = np.arange(64)[:, None]
    t_ = np.arange(64)[None, :]
    c2 = np.cos(2 * np.pi * t_ * r_ / 64)
    s2m = np.sin(2 * np.pi * t_ * r_ / 64)
    w2 = np.stack([c2, -s2m], 1).astype(np.float32)       # [64, 2, 64]
    ctw = np.cos(2 * np.pi * q * r_ / D)
    stw = np.sin(2 * np.pi * q * r_ / D)
    tw = np.stack([ctw, stw], 1).astype(np.float32)       # [64, 2, 128]
    ident = np.eye(128, dtype=np.float32)
    return amat, gp, _round_fp32r(w1), _round_fp32r(w2), tw, ident


_NC_CACHE = None
LAST_RESULTS = None


def kernel(bottom1, bottom2, rand_s_1, rand_s_2, rand_h_1, rand_h_2):
    global _NC_CACHE
    if _NC_CACHE is None:
        _NC_CACHE = _build_nc()
    nc = _NC_CACHE

    amat, gp, w1, w2, tw, ident = _host_consts(
        np.asarray(rand_s_1), np.asarray(rand_s_2),
        np.asarray(rand_h_1), np.asarray(rand_h_2))

    x1 = np.asarray(bottom1, np.float32).reshape(B, HW, C)
    x2 = np.asarray(bottom2, np.float32).reshape(B, HW, C)

    in_maps = []
    for core in range(NCORES):
        bs = slice(core * BPC, (core + 1) * BPC)
        xt = np.zeros((2, C, ROWS_PAD), np.float32)
        xt[0, :, :ROWS] = x1[bs].reshape(ROWS, C).T
        xt[1, :, :ROWS] = x2[bs].reshape(ROWS, C).T
        # layout [p, inp, cc, rt, 128]
        xt = np.ascontiguousarray(
            xt.reshape(2, CCN, 128, RT, 128).transpose(2, 3, 0, 1, 4))
        xt = _round_fp32r(xt)
        in_maps.append({
            "xt": xt, "amat": amat, "gpool": gp,
            "w1": w1, "w2": w2, "tw": tw, "ident": ident,
        })

    res = run_bass_kernel_spmd(nc, in_maps, core_ids=list(range(NCORES)))
    global LAST_RESULTS
    LAST_RESULTS = res
    out = np.concatenate([res.results[c]["out"] for c in range(NCORES)], 0)
    return out.astype(np.float32)


if __name__ == "__main__":
    rng = np.random.default_rng(0)
    b1 = rng.standard_normal((B, Hh, Ww, C)).astype(np.float32)
    b2 = rng.standard_normal((B, Hh, Ww, C)).astype(np.float32)
    s1 = (2.0 * rng.integers(0, 2, C) - 1.0).astype(np.float32)
    s2 = (2.0 * rng.integers(0, 2, C) - 1.0).astype(np.float32)
    h1 = rng.integers(0, D, C, dtype=np.int32)
    h2 = rng.integers(0, D, C, dtype=np.int32)
    out = kernel(bottom1=b1, bottom2=b2, rand_s_1=s1, rand_s_2=s2,
                 rand_h_1=h1, rand_h_2=h2)
    print(out.shape, out.dtype)
